# revision 33
# baseline (speedup 1.0000x reference)
"""ConceptContrastiveLoss Trainium2 kernel (8-core SPMD, batch-parallel,
per-core pairwise partials, one exposed collective).

Per core (64 MiB of input): each batch item [2048, 128] is DMA'd as one
contiguous 1 MiB transfer into SBUF [128 x 2048] (16 seq rows per
partition), tree-halved down to 128 elems by exact fp32 adds -- batches
alternating between DVE and Pool -- then one fp32 ones-matmul on TensorE
folds the partitions into a D-major raw-sum centroid column in PSUM.
The 1/S centroid scale is folded into the pairwise weights (everything
is quadratic in the centroids).

Cross-core structure (v1 exposed a full V-centroid AllGather plus the
whole EV tail behind it; here the only exposed exchange is a [128,4]
payload):
- E half first.  Local E centroids bounce to DRAM and AllGather on the
  gpsimd queue; the ~18.5us collective (15us cost-model constant) plus
  the CtE reload hide under the V bulk half.
- All E-dependent prep (m2E, sqE, E norms for the K=64 aug matmul, EE
  clustering closed form B*sum|e|^2-|sum e|^2) is emitted between V
  bulk groups so it executes mid-bulk in engine-queue order, hidden.
- EV separation is computed as per-core partials: hinge terms for
  (all 256 E) x (32 local V centroids), in two chunks -- V cols 0:24
  mid-bulk (hidden), cols 24:32 on the tail.  VV clustering uses the
  closed form, whose ingredients (sum v, sum|v|^2) are linear per-core
  partials.  The closed form yields the i<j sum directly, so the
  clustering weights are BETA/n_pairs and GAMMA/n_pairs (v1 halved the
  VV term; masked by the hinge dominating the loss, ~6.7e-5 rel err --
  this kernel measures 1.1e-7).
- Tail: pack [hinge, sum|v|^2, sum v, pad] per-partition partials into
  [128,4], bounce, AllGather (the ONE exposed collective), reload split
  across both HWDGE queues, tree-sum the 8 core blocks, combine with
  the precomputed EE column, fold across partitions with a ones-matmul,
  DMA the scalar out.

MultiCoreSim (the cost model the harness's "HW exec time" tracks):
v1 graded 200092 ns / simmed 194380; the prior overlap kernel simmed
140478; this kernel sims at 136529 ns against a 104073 ns pure-DMA
floor (64 MiB/core HBM read serialized on the shared DMA-engine
device).  Verified on real TRN2 via run_bass_kernel_spmd: rel err
1.092e-07.

A remote_dma_broadcast variant (SBUF->SBUF XOR recursive-doubling
exchanges, no collectives) simmed at 115583 ns and its data transfers
verify on real hardware (the logical->physical core map is [0,1,2,3,
6,7,4,5]; XOR-relative routing stays bijective so permutation-invariant
sums are unaffected), but remote-SEMAPHORE delivery never fires on this
runtime -- every receive-side wait hangs the device (the wait machinery
itself is fine: this kernel's sim matches the same cost model).  Parked
until the rsem path works; see git/file history for the full variant.

Build knobs: probe="dma" (bulk DMAs only, HBM floor) / probe="bulk";
solo=True replaces collectives with local-DMA stand-ins (for full_loop
timing: NRT forbids collectives inside hardware loops).
"""

from contextlib import ExitStack, nullcontext

import numpy as np

import concourse.bacc as bacc
import concourse.bass as bass
import concourse.mybir as mybir
import concourse.tile as tile
from concourse.bass_utils import run_bass_kernel_spmd
from concourse.tile import add_dep_helper

F32 = mybir.dt.float32

MARGIN = 10.0
ALPHA = 3.0
BETA = 0.3
GAMMA = 0.3

B, S, D = 256, 2048, 128
N_CORES = 8
BPD = 1   # batch items per DMA (DMA size = BPD MiB)
BUFS = 16  # big-tile pool buffers
DVE_STOP = 128       # halving-tree handoff width (elems); PE folds the rest
N_DMA_ENG = 2        # DMA issue streams: 2 = SP+ACT HWDGE
RED_ASSIGN = "vg"   # per-batch seq-reduce engine cycle: v=DVE, g=Pool


def _patch_sim_nc_mappings():
    """fake_nrt (sim-only environments) lacks the logical->physical nc-map
    ioctls that MultiCoreSim's remote-DMA routing uses; identity-map them.
    On real hardware the driver call succeeds and nothing is patched."""
    import concourse.libnrt as libnrt

    try:
        libnrt.get_trn2_nc_mapping()
        libnrt.get_device_id_to_routing_id_mapping()
        return
    except Exception:
        pass
    ident_nc = {(d, i): i for d in range(64) for i in range(8)}
    rid_map = {d: d for d in range(64)}
    libnrt.get_trn2_nc_mapping = lambda: ident_nc
    libnrt.get_device_id_to_routing_id_mapping = lambda: rid_map
    try:
        import concourse.bass_interp as bass_interp

        bass_interp.get_device_id_to_routing_id_mapping = lambda: rid_map
    except Exception:
        pass


_patch_sim_nc_mappings()


def _inject_waits(nc, waits):
    """waits: {inst_name: [(sem_num, sem_name, value), ...]} appended as
    encoded semaphore waits on the named instructions, post tile-scheduling
    (the scheduling pass single-core-simulates and would deadlock on waits
    only a remote core satisfies)."""
    fn = nc.m.functions[0]
    found = set()
    for blk in fn.blocks:
        for ins in blk.instructions:
            if ins.name in waits:
                nws = [
                    mybir.SyncWait(
                        sync_type="semaphore", id=sem_num, ant_name=sem_name,
                        wait_mode="sem-ge-imm", wait_value=val,
                    )
                    for sem_num, sem_name, val in waits[ins.name]
                ]
                si = ins.sync_info
                if si is None:
                    ins.sync_info = mybir.SyncInfo(on_wait=nws, on_update=[])
                else:
                    si.on_wait = list(si.on_wait) + nws
                found.add(ins.name)
    missing = set(waits) - found
    assert not missing, f"wait-injection targets not found: {missing}"


def _resolve_tick_waits(nc, tick_waits, waits):
    """tick_waits: {consumer_inst: producer_inst}.  For each producer, find
    its engine-completion sem (the tile-assigned monotonic '<ENG>_nn' ++1
    update) and the cumulative count of that sem up to and including the
    producer in scheduled order; append (sem, count) as a wait on the
    consumer.  Avoids adding updates to the producer (the sim caps
    updates-per-instruction)."""
    import re

    fn = nc.m.functions[0]
    prod_names = {p for p in tick_waits.values()}
    counts = {}
    prod_tick = {}
    eng_re = re.compile(r"^(DVE|Pool|PE|SP|Activation)_")
    for blk in fn.blocks:
        for ins in blk.instructions:
            si = ins.sync_info
            ups = list(si.on_update) if si is not None else []
            for u in ups:
                if u.sync_type != "semaphore":
                    continue
                v = u.update_value if u.update_value is not None else 1
                counts[u.id] = counts.get(u.id, 0) + v
            if ins.name in prod_names:
                eng_ups = [
                    u for u in ups
                    if u.sync_type == "semaphore"
                    and u.ant_name and eng_re.match(u.ant_name)
                ]
                assert eng_ups, (
                    f"{ins.name}: no engine-sem update to tick against"
                )
                u = eng_ups[0]
                prod_tick[ins.name] = (u.id, u.ant_name, counts[u.id])
    for consumer, producer in tick_waits.items():
        assert producer in prod_tick, f"producer {producer} not found"
        waits.setdefault(consumer, []).append(prod_tick[producer])


def _bcast_dests(delta):
    # one relative dest (0, delta); cross-die dests (tpb bit 2) must occupy
    # a D2D-capable slot (4-7)
    rd = [None] * 8
    rd[4 if (delta & 4) else 0] = (0, delta)
    return rd


def _build_body(tc, e, v, out, loc_cE, gathE, loc_p, gath_p, B, S, D,
                n_cores,
                solo=False, bpd=BPD, bufs=BUFS, loop_r=1, dve_stop=DVE_STOP,
                n_dma_eng=N_DMA_ENG, full_loop=False, probe=None,
                red_assign=RED_ASSIGN, waits=None, tick_waits=None):
    nc = tc.nc
    Bl = B // n_cores  # local batches per tensor
    J = S // 128       # seq tiles per batch item
    n_pairs = B * (B - 1) // 2
    w_ev = ALPHA / (B * B)
    w_ee = BETA / n_pairs   # closed form yields the i<j sum directly
    w_vv = GAMMA / n_pairs
    assert Bl == 32 and D == 128 and dve_stop == D
    GRP = 8
    n_groups = Bl // GRP

    with ExitStack() as ctx:
        # ---- pools ----
        consts = ctx.enter_context(tc.tile_pool(name="consts", bufs=1))
        big_pool = ctx.enter_context(tc.tile_pool(name="big", bufs=bufs))
        cps = ctx.enter_context(tc.tile_pool(name="cps", bufs=2, space="PSUM"))
        sp = ctx.enter_context(tc.tile_pool(name="sp", bufs=1))
        sps = ctx.enter_context(tc.tile_pool(name="sps", bufs=1, space="PSUM"))
        spp = ctx.enter_context(tc.tile_pool(name="spp", bufs=2, space="PSUM"))
        spf = ctx.enter_context(tc.tile_pool(name="spf", bufs=1, space="PSUM"))
        trash_pool = ctx.enter_context(tc.tile_pool(name="trash", bufs=2))

        ones_col = nc.const_aps.aps[(F32, 1.0)]
        b_eps = consts.tile([128, 1], F32, name="b_eps")
        nc.vector.memset(b_eps[:], 1e-12)
        b_margin = consts.tile([128, 1], F32, name="b_margin")
        nc.vector.memset(b_margin[:], MARGIN)
        fin = sp.tile([1, 1], F32, name="fin")

        assert bpd == 1
        assert not (full_loop and not solo), "hardware loops require solo"
        do_x = probe not in ("dma", "bulk")
        do_coll = do_x and not full_loop

        # ---- cross-core exchange state ----
        # CtE accumulates the full 256 E centroids (local 32 + 3 gather
        # rounds); pay/rcv are the tail allreduce ping-pong tiles.
        CtE = sp.tile([D, B], F32, name="CtE")
        pay = [sp.tile([128, 4], F32, name=f"pay{k}") for k in range(4)]
        rcv = [sp.tile([128, 4], F32, name=f"rcvF{k}") for k in range(3)]

        # E-derived tiles (all hidden under V bulk)
        m2E = sp.tile([D, B], F32, name="m2E")
        sqE = sp.tile([D, B], F32, name="sqE")
        ag_e = sp.tile([64, B], F32, name="ag_e")    # row0: |e|^2, row32: 1
        rhs_v = sp.tile([64, Bl], F32, name="rhs_v")  # row0: 1, row32: |v|^2
        EEcol = sp.tile([128, 1], F32, name="EEcol")
        eQ = sp.tile([128, 1], F32, name="eQ")
        eS = sp.tile([128, 1], F32, name="eS")
        e2 = sp.tile([128, 1], F32, name="e2")

        # V-local tiles
        CtVl = sp.tile([D, Bl], F32, name="CtVl")
        sqV = sp.tile([D, Bl], F32, name="sqV")
        accV = sp.tile([128, 4], F32, name="accV")  # 2 chunks x 2 E-blocks

        nc.vector.memset(ag_e[:], 0.0)
        nc.vector.memset(ag_e[32:33, :], 1.0)
        nc.vector.memset(rhs_v[:], 0.0)
        nc.vector.memset(rhs_v[0:1, :], 1.0)
        nc.vector.memset(pay[0][:, 3:4], 0.0)

        g_half = [None, None]
        dma_engines = [nc.sync, nc.scalar, nc.gpsimd][:n_dma_eng]
        dma_i = [0]
        red_i = [0]

        last_pool_red = [None]  # last Pool-assigned bulk reduce instruction

        def emit_bulk_batch(src, b0, G, col):
            Tb = big_pool.tile([128, bpd * J * D], F32, name="Tb")
            eng = dma_engines[dma_i[0] % len(dma_engines)]
            dma_i[0] += 1
            eng.dma_start(
                out=Tb[:], in_=src[b0].rearrange("(p j) d -> p (j d)", p=128)
            )
            if probe == "dma":
                return None
            # last 4 batches forced to DVE so Pool's bulk work ends early
            # enough to hide the tail-allreduce desc-gen behind bulk DMA
            ch = "v" if red_i[0] >= 2 * B // n_cores - 4 else (
                red_assign[red_i[0] % len(red_assign)]
            )
            red_eng = {"v": nc.vector, "g": nc.gpsimd}[ch]
            red_i[0] += 1
            w = J * D // 2
            red = None
            while w >= dve_stop:
                red = red_eng.tensor_add(
                    Tb[:, 0:w], Tb[:, 0:w], Tb[:, w : 2 * w]
                )
                w //= 2
            if ch == "g":
                last_pool_red[0] = red
            return nc.tensor.matmul(
                out=G[:, col : col + 1],
                lhsT=Tb[:, 0:D],
                rhs=ones_col,
                start=(col % GRP == 0),
                stop=(col % GRP == GRP - 1),
            )

        def emit_v_group_capture(G, gl):
            # PSUM -> SBUF grab of the group's raw centroids + squared copy
            c0 = gl * GRP
            nc.vector.tensor_copy(CtVl[:, c0 : c0 + GRP], G[:, c0 : c0 + GRP])
            nc.vector.scalar_tensor_tensor(
                sqV[:, c0 : c0 + GRP], CtVl[:, c0 : c0 + GRP], 1.0 / (S * S),
                CtVl[:, c0 : c0 + GRP],
                op0=mybir.AluOpType.mult, op1=mybir.AluOpType.mult,
            )

        psn = None

        def emit_hinge_chunk(ci, c0, cn):
            # EV hinge terms for (all E) x (local V cols c0:c0+cn), partials
            # accumulated per-partition into accV cols 2*ci + blk.
            nc.tensor.matmul(
                out=psn[32:33, c0 : c0 + cn], lhsT=ones_col,
                rhs=sqV[:, c0 : c0 + cn],
            )
            nc.vector.tensor_copy(
                rhs_v[32:33, c0 : c0 + cn], psn[32:33, c0 : c0 + cn]
            )
            for bi, bs in enumerate(range(0, B, 128)):
                Pb = spp.tile([128, 512], F32, name="P_ev", tag="P_ev")
                P = Pb[:, 0:cn]
                nc.tensor.matmul(
                    out=P, lhsT=m2E[:, bs : bs + 128],
                    rhs=CtVl[:, c0 : c0 + cn], start=True, stop=False,
                )
                nc.tensor.matmul(
                    out=P, lhsT=ag_e[:, bs : bs + 128],
                    rhs=rhs_v[:, c0 : c0 + cn], start=False, stop=True,
                )
                dist = trash_pool.tile([128, Bl], F32, name="dist", tag="rel")
                hin = trash_pool.tile([128, Bl], F32, name="hin", tag="rel")
                hsq = trash_pool.tile([128, Bl], F32, name="hsq", tag="rel")
                nc.vector.tensor_scalar_max(P, P, 0.0)
                nc.scalar.activation(
                    dist[:, 0:cn], P, mybir.ActivationFunctionType.Sqrt,
                    bias=b_eps[:],
                )
                nc.scalar.activation(
                    hin[:, 0:cn], dist[:, 0:cn],
                    mybir.ActivationFunctionType.Relu,
                    bias=b_margin[:], scale=-1.0,
                )
                col = 2 * ci + bi
                nc.scalar.activation(
                    hsq[:, 0:cn], hin[:, 0:cn],
                    mybir.ActivationFunctionType.Square,
                    accum_out=accV[:, col : col + 1],
                )

        loop_cm = tc.For_i(0, loop_r, 1) if full_loop else nullcontext()
        with loop_cm:
            # ================= E half =================
            if probe != "dma":
                g_half[0] = cps.tile([128, 512], F32, name="GaccE")
            for b0 in range(Bl):
                emit_bulk_batch(e, b0, g_half[0], b0)
            dmaE = None
            if probe != "dma":
                cpE = nc.vector.tensor_copy(CtE[:, 0:Bl], g_half[0][:, 0:Bl])
                if do_coll:
                    # bounce local E centroids to DRAM and AllGather them;
                    # the ~18.5us collective hides under the V bulk half.
                    # gpsimd queue: SEQ frees before the collective delay.
                    nc.gpsimd.dma_start(out=loc_cE[:], in_=CtE[:, 0:Bl])
                    if solo:
                        nc.gpsimd.dma_start(out=gathE[0:D, :], in_=loc_cE[:])
                    else:
                        nc.gpsimd.collective_compute(
                            "AllGather",
                            mybir.AluOpType.bypass,
                            replica_groups=[list(range(n_cores))],
                            ins=[loc_cE[:]],
                            outs=[gathE[:]],
                        )
                    # reload the full centroid set; gpsimd so no bulk DMA
                    # issue queue ever stalls behind the collective wait
                    dmaE = nc.gpsimd.dma_start(
                        out=CtE[:].rearrange("p (c j) -> p c j", c=n_cores),
                        in_=gathE.rearrange("(c p) j -> p c j", c=n_cores),
                    )
                elif do_x:
                    # full_loop stand-in: fabricate the remote columns
                    nc.vector.memset(CtE[:, Bl:B], 1.0)

            # ================= V half =================
            if probe != "dma":
                g_half[1] = cps.tile([128, 512], F32, name="GaccV")
            for gl in range(n_groups):
                for ti in range(GRP):
                    b0 = gl * GRP + ti
                    emit_bulk_batch(v, b0, g_half[1], b0)
                if do_x:
                    emit_v_group_capture(g_half[1], gl)

                if gl == 0 and do_x:
                    nc.vector.memset(accV[:], 0.0)
                    # --- E-dependent prep (executes mid-bulk, after the
                    # E-gather rounds land ~56us; V bulk runs to ~107us).
                    # Every direct CtE reader gets the round-2 wait: the
                    # scheduler may order them arbitrarily among themselves.
                    mul = nc.vector.tensor_scalar_mul(
                        m2E[:], CtE, -2.0 * (1.0 / (S * S))
                    )
                    sqe = nc.vector.scalar_tensor_tensor(
                        sqE[:], CtE, (1.0 / (S * S)), CtE,
                        op0=mybir.AluOpType.mult, op1=mybir.AluOpType.mult,
                    )
                    psn = sps.tile([128, 512], F32, name="psn", tag="psn")
                    nc.tensor.matmul(
                        out=psn[0:1, 0:B], lhsT=ones_col, rhs=sqE[:]
                    )
                    nc.vector.tensor_copy(ag_e[0:1, :], psn[0:1, 0:B])
                    # EE clustering closed form (identical on every core)
                    eqr = nc.vector.reduce_sum(
                        eQ[:], sqE[:], axis=mybir.AxisListType.X
                    )
                    esr = nc.vector.reduce_sum(
                        eS[:], CtE, axis=mybir.AxisListType.X
                    )
                    nc.vector.scalar_tensor_tensor(
                        e2[:], eS[:], 1.0 / (S * S), eS[:],
                        op0=mybir.AluOpType.mult, op1=mybir.AluOpType.mult,
                    )
                    nc.vector.scalar_tensor_tensor(
                        EEcol[:], eQ[:], float(B), e2[:],
                        op0=mybir.AluOpType.mult, op1=mybir.AluOpType.subtract,
                    )

                if gl == 2 and do_x:
                    # hinge chunk A: V cols 0:24 (hidden under group-3 bulk)
                    emit_hinge_chunk(0, 0, 3 * GRP)

            if probe in ("dma", "bulk"):
                nc.vector.memset(fin[:], 0.0)
                nc.sync.dma_start(out=out[:], in_=fin[:])
                return

            # ================= exposed tail =================
            # hinge chunk B: V cols 24:32
            emit_hinge_chunk(1, 3 * GRP, GRP)

            # pack per-partition partials: [hinge, sum|v|^2, sum v, pad]
            nc.vector.reduce_sum(pay[0][:, 0:1], accV[:], axis=mybir.AxisListType.X)
            nc.vector.reduce_sum(pay[0][:, 1:2], sqV[:], axis=mybir.AxisListType.X)
            pk = nc.vector.reduce_sum(
                pay[0][:, 2:3], CtVl[:], axis=mybir.AxisListType.X
            )

            # exchange the [128,4] per-core partials: bounce -> AllGather
            # (the one exposed collective: 15us constant + ~0.4us) ->
            # reload -> tree-sum the 8 core blocks
            last = pay[0]
            if do_coll:
                nc.sync.dma_start(out=loc_p[:], in_=pay[0][:])
                if solo:
                    dgp = nc.gpsimd.dma_start(
                        out=gath_p[0:128, :], in_=loc_p[:]
                    )
                else:
                    dgp = nc.gpsimd.collective_compute(
                        "AllGather",
                        mybir.AluOpType.bypass,
                        replica_groups=[list(range(n_cores))],
                        ins=[loc_p[:]],
                        outs=[gath_p[:]],
                    )
                P8 = sp.tile([128, 8 * 4], F32, name="P8")
                h = n_cores // 2
                nc.sync.dma_start(
                    out=P8[:, 0 : 4 * h].rearrange("p (c j) -> p c j", c=h),
                    in_=gath_p[0 : h * 128].rearrange("(c p) j -> p c j", c=h),
                )
                nc.scalar.dma_start(
                    out=P8[:, 4 * h :].rearrange("p (c j) -> p c j", c=h),
                    in_=gath_p[h * 128 :].rearrange("(c p) j -> p c j", c=h),
                )
                t16 = sp.tile([128, 16], F32, name="t16")
                t8 = sp.tile([128, 8], F32, name="t8")
                nc.vector.tensor_add(t16[:], P8[:, 0:16], P8[:, 16:32])
                nc.vector.tensor_add(t8[:], t16[:, 0:8], t16[:, 8:16])
                nc.vector.tensor_add(pay[3][:], t8[:, 0:4], t8[:, 4:8])
                last = pay[3]

            # combine: tot = w_ev*hinge + w_ee*EE + w_vv*(B*q - |s|^2/S^2)
            vv2 = sp.tile([128, 1], F32, name="vv2")
            tvv = sp.tile([128, 1], F32, name="tvv")
            tot = sp.tile([128, 1], F32, name="tot")
            nc.vector.scalar_tensor_tensor(
                vv2[:], last[:, 2:3], 1.0 / (S * S), last[:, 2:3],
                op0=mybir.AluOpType.mult, op1=mybir.AluOpType.mult,
            )
            nc.vector.scalar_tensor_tensor(
                tvv[:], last[:, 1:2], float(B), vv2[:],
                op0=mybir.AluOpType.mult, op1=mybir.AluOpType.subtract,
            )
            nc.vector.tensor_scalar_mul(tot[:], last[:, 0:1], w_ev)
            nc.vector.scalar_tensor_tensor(
                tot[:], EEcol[:], w_ee, tot[:],
                op0=mybir.AluOpType.mult, op1=mybir.AluOpType.add,
            )
            nc.vector.scalar_tensor_tensor(
                tot[:], tvv[:], w_vv, tot[:],
                op0=mybir.AluOpType.mult, op1=mybir.AluOpType.add,
            )
            psFb = spf.tile([128, 512], F32, name="psF", tag="psF")
            psF = psFb[0:1, 0:1]
            nc.tensor.matmul(out=psF, lhsT=ones_col, rhs=tot[:])
            nc.scalar.copy(fin[:], psF)
            nc.sync.dma_start(out=out[:], in_=fin[:])


def build_nc(B=B, S=S, D=D, n_cores=N_CORES, solo=False, bpd=None, bufs=None,
             loop_r=1, dve_stop=None, n_dma_eng=None, full_loop=False,
             probe=None, red_assign=None, overlap=None):
    Bl = B // n_cores
    nc = bacc.Bacc("TRN2", num_devices=n_cores, num_swdge_queues=4)
    e = nc.dram_tensor("expert_concepts", [Bl, S, D], F32, kind="ExternalInput").ap()
    v = nc.dram_tensor("violator_concepts", [Bl, S, D], F32, kind="ExternalInput").ap()
    out = nc.dram_tensor("out", [1, 1], F32, kind="ExternalOutput").ap()
    gspace = "Local" if solo else "Shared"
    loc_cE = nc.dram_tensor("loc_cE", [D, Bl], F32).ap()
    gathE = nc.dram_tensor("gathE", [n_cores * D, Bl], F32,
                           addr_space=gspace).ap()
    loc_p = nc.dram_tensor("loc_p", [128, 4], F32).ap()
    gath_p = nc.dram_tensor("gath_p", [n_cores * 128, 4], F32,
                            addr_space=gspace).ap()
    waits = {}
    tick_waits = {}
    with tile.TileContext(nc) as tc:
        _build_body(
            tc, e, v, out, loc_cE, gathE, loc_p, gath_p, B, S, D, n_cores,
            solo=solo,
            bpd=bpd if bpd is not None else BPD,
            bufs=bufs if bufs is not None else BUFS,
            loop_r=loop_r,
            dve_stop=dve_stop if dve_stop is not None else DVE_STOP,
            n_dma_eng=n_dma_eng if n_dma_eng is not None else N_DMA_ENG,
            full_loop=full_loop,
            probe=probe,
            red_assign=red_assign if red_assign is not None else RED_ASSIGN,
            waits=waits,
            tick_waits=tick_waits,
        )
    nc.compile()
    if tick_waits:
        _resolve_tick_waits(nc, tick_waits, waits)
    if waits:
        _inject_waits(nc, waits)
    return nc


def _run(expert_concepts, violator_concepts, **spmd_kwargs):
    expert_concepts = np.ascontiguousarray(expert_concepts, dtype=np.float32)
    violator_concepts = np.ascontiguousarray(violator_concepts, dtype=np.float32)
    assert expert_concepts.shape == (B, S, D)
    assert violator_concepts.shape == (B, S, D)

    nc = build_nc()
    Bl = B // N_CORES
    in_maps = [
        {
            "expert_concepts": expert_concepts[c * Bl : (c + 1) * Bl],
            "violator_concepts": violator_concepts[c * Bl : (c + 1) * Bl],
        }
        for c in range(N_CORES)
    ]
    res = run_bass_kernel_spmd(nc, in_maps, list(range(N_CORES)), **spmd_kwargs)
    return np.float32(res.results[0]["out"][0, 0]), res


def kernel(expert_concepts: np.ndarray, violator_concepts: np.ndarray) -> np.ndarray:
    out, _ = _run(expert_concepts, violator_concepts)
    return out
